# revision 15
# baseline (speedup 1.0000x reference)
"""MoE routing kernel for Trainium2, 8 NeuronCores, SPMD.

Strategy: token-parallel across cores (2048 tokens/core); each core runs the
router in fp32 (selection is numerically chaotic -- probs are near-uniform, so
the router matmul/distances must be fp32; monotone transforms are skipped where
they don't affect ordering), then computes only the routed (token, expert)
pairs: per expert a static-capacity (384) compacted token list is built on
device (matmul cumsum + indirect scatter), tokens are gathered with
dma_gather (transposed + row-major bf16), expert FFN runs in bf16 on the
tensor engine, and per-token contributions are combined token-major.
"""

import sys
import numpy as np

sys.path.insert(0, "/opt/trn_rl_repo")

import ml_dtypes

import concourse.bass as bass
import concourse.mybir as mybir
import concourse.tile as tile
from concourse import bacc
from concourse.bass_utils import run_bass_kernel_spmd
from concourse.masks import make_identity, make_upper_triangular

F32 = mybir.dt.float32
BF16 = mybir.dt.bfloat16
I32 = mybir.dt.int32
I16 = mybir.dt.int16
AF = mybir.ActivationFunctionType
OP = mybir.AluOpType

N, H, C, E, R = 16384, 1024, 256, 16, 128
NCORES = 8
NT = N // NCORES          # tokens per core = 2048
TILES = NT // 128         # 16 token tiles
CAP = 384                 # per-expert capacity (measured max 339)
CAPT = CAP // 128         # 3 slot tiles
HC = H // 128             # 8 hidden chunks
PADROW = NT               # trash row id in padded xb
NPOOL = E * CAP           # 6144

_BUILT = None


def _build():
    nc = bacc.Bacc("TRN2", target_bir_lowering=False, debug=False,
                   num_devices=NCORES)

    xs_d = nc.dram_tensor("xs", [NT, H], F32, kind="ExternalInput")
    xT_d = nc.dram_tensor("xT", [H, NT], F32, kind="ExternalInput")
    xb_d = nc.dram_tensor("xb", [NT + 128, H], BF16, kind="ExternalInput")
    wd_d = nc.dram_tensor("Wd", [H, C], F32, kind="ExternalInput")
    bd_d = nc.dram_tensor("bd", [C], F32, kind="ExternalInput")
    cen_d = nc.dram_tensor("cen", [E, C], F32, kind="ExternalInput")
    wg_d = nc.dram_tensor("Wgb", [E, H, H], BF16, kind="ExternalInput")
    bg_d = nc.dram_tensor("bg", [E, H], F32, kind="ExternalInput")
    u_d = nc.dram_tensor("Ub", [E, H, R], BF16, kind="ExternalInput")
    v_d = nc.dram_tensor("Vb", [E, R, H], BF16, kind="ExternalInput")

    out_d = nc.dram_tensor("out", [NT, H], F32, kind="ExternalOutput")
    dprob_d = nc.dram_tensor("dbg_probs", [NT, E], F32, kind="ExternalOutput")
    dslot_d = nc.dram_tensor("dbg_slots", [NT, 2], I32, kind="ExternalOutput")

    with tile.TileContext(nc) as tc:
        import contextlib
        ctx = contextlib.ExitStack()
        with ctx:
            cpool = ctx.enter_context(tc.tile_pool(name="const", bufs=1))
            spool = ctx.enter_context(tc.tile_pool(name="state", bufs=1))
            dram = ctx.enter_context(tc.tile_pool(name="dram", bufs=1,
                                                  space="DRAM"))

            # ---------------- constants ----------------
            ident = cpool.tile([128, 128], F32)
            make_identity(nc, ident[:])
            ut128 = cpool.tile([128, 128], F32)       # 1 where p <= i
            make_upper_triangular(nc, ut128[:], val=1.0, diag=True)
            sut128 = cpool.tile([128, 128], F32)      # 1 where p < i
            make_upper_triangular(nc, sut128[:], val=1.0, diag=False)
            ones_k = cpool.tile([128, 1], F32)
            nc.vector.memset(ones_k[:], 1.0)
            ones_r = cpool.tile([1, 128], F32)
            nc.vector.memset(ones_r[:], 1.0)
            ones_rb = cpool.tile([1, 128], BF16)
            nc.vector.memset(ones_rb[:], 1.0)
            big = cpool.tile([128, TILES, E], F32)
            nc.vector.memset(big[:], 8.0e6)
            bigi = cpool.tile([128, TILES, E], I32)
            nc.vector.memset(bigi[:], 8000000)
            ebase_i = cpool.tile([128, TILES, E], I32)    # e*CAP per (i,e)
            nc.gpsimd.iota(ebase_i[:], pattern=[[0, TILES], [CAP, E]],
                           base=0, channel_multiplier=0)
            ids16 = cpool.tile([128, TILES], I16)     # token id = i*128+p
            nc.gpsimd.iota(ids16[:], pattern=[[128, TILES]], base=0,
                           channel_multiplier=1)

            # persistent state across phases
            sx_sb = spool.tile([128, TILES, H], F32)       # s * x rows
            w01 = spool.tile([128, TILES, 2], F32)         # top1/top2 probs
            slots_i = spool.tile([128, TILES, 2], I32)     # z-pool row per rank
            probs_all = spool.tile([128, TILES, E], F32)

            idxp = dram.tile([NPOOL, 1], I16)              # wrapped idx pool
            zp = dram.tile([NPOOL, H], BF16)               # contribution pool

            # zero the idx pool (pad slots must hold a valid index)
            zini = cpool.tile([128, NPOOL // 128], I16)
            nc.vector.memset(zini[:], 0)
            nc.sync.dma_start(
                idxp[:].rearrange("(p s) o -> p (s o)", p=128), zini[:])

            # =============== ROUTER (fp32) ===============
            with tc.tile_pool(name="rt", bufs=1) as rp, \
                 tc.tile_pool(name="rtd", bufs=1) as rtd:
                xtp_cm = tc.tile_pool(name="xtp", bufs=1)
                xtp = xtp_cm.__enter__()
                xT_sb = xtp.tile([128, HC, NT], F32)
                nc.sync.dma_start(
                    xT_sb[:], xT_d[:].rearrange("(hc p) t -> p hc t", p=128))
                wd_sb = rp.tile([128, HC, C], F32, tag="wd")
                nc.sync.dma_start(
                    wd_sb[:], wd_d[:].rearrange("(hc p) c -> p hc c", p=128))
                bdT = rp.tile([128, 2], F32, tag="bd")
                nc.sync.dma_start(bdT[:],
                                  bd_d[:].rearrange("(cc p) -> p cc", p=128))

                # centroid normalization -> cnT [128(c), 2, 16]
                cen_sb = rp.tile([16, C], F32, tag="cen")
                nc.sync.dma_start(cen_sb[:], cen_d[:])
                csq = rp.tile([16, C], F32, tag="csq")
                nc.vector.tensor_tensor(csq[:], cen_sb[:], cen_sb[:], OP.mult)
                cns = rp.tile([16, 1], F32, tag="cns")
                nc.vector.tensor_reduce(cns[:], csq[:], mybir.AxisListType.X,
                                        OP.add)
                cnrt = rp.tile([16, 1], F32, tag="cnrt")
                nc.scalar.activation(cnrt[:], cns[:], AF.Sqrt)
                cnr = rp.tile([16, 1], F32, tag="cnr")
                nc.vector.reciprocal(cnr[:], cnrt[:])
                cnpad = rp.tile([128, C], F32, tag="cnp")
                nc.vector.memset(cnpad[:], 0.0)
                nc.vector.tensor_scalar_mul(cnpad[:16, :], cen_sb[:], cnr[:])
                ppA_cm = tc.tile_pool(name="ppA", bufs=2, space="PSUM")
                ppA = ppA_cm.__enter__()
                cnT = rp.tile([128, 2, 16], F32, tag="cnT")
                for cc in range(2):
                    tp = ppA.tile([128, 128], F32, tag="ctp")
                    nc.tensor.transpose(tp[:], cnpad[:, cc * 128:(cc + 1) * 128],
                                        ident[:])
                    nc.vector.tensor_copy(cnT[:, cc, :], tp[:, :16])

                # distilled^T = gelu(Wd^T x^T + bd)  [128(c), 2, NT]
                distT = rtd.tile([128, 2, NT], F32)
                for cc in range(2):
                    for tt in range(NT // 512):
                        dp = ppA.tile([128, 512], F32, tag="dp")
                        for hc in range(HC):
                            nc.tensor.matmul(
                                dp[:],
                                lhsT=wd_sb[:, hc, cc * 128:(cc + 1) * 128],
                                rhs=xT_sb[:, hc, tt * 512:(tt + 1) * 512],
                                start=(hc == 0), stop=(hc == HC - 1))
                        nc.scalar.activation(
                            distT[:, cc, tt * 512:(tt + 1) * 512], dp[:],
                            AF.Gelu, bias=bdT[:, cc:cc + 1])

                ppA_cm.__exit__(None, None, None)
                xtp_cm.__exit__(None, None, None)

                # dists token-major [128(t), TILES, 16]
                distTM = spool.tile([128, TILES, E], F32)
                ppB_cm = tc.tile_pool(name="ppB", bufs=1, space="PSUM")
                ppB = ppB_cm.__enter__()
                ppB2_cm = tc.tile_pool(name="ppB2", bufs=2, space="PSUM")
                ppB2 = ppB2_cm.__enter__()
                for tt in range(NT // 512):
                    nsq = ppB.tile([1, 512], F32, tag="nsq")
                    dots = ppB.tile([16, 512], F32, tag="dots")
                    for cc in range(2):
                        sqc = rp.tile([128, 512], F32, tag="sqc")
                        nc.vector.tensor_tensor(
                            sqc[:], distT[:, cc, tt * 512:(tt + 1) * 512],
                            distT[:, cc, tt * 512:(tt + 1) * 512], OP.mult)
                        nc.tensor.matmul(
                            nsq[:], lhsT=ones_k[:],
                            rhs=sqc[:],
                            start=(cc == 0), stop=(cc == 1))
                        nc.tensor.matmul(
                            dots[:], lhsT=cnT[:, cc, :],
                            rhs=distT[:, cc, tt * 512:(tt + 1) * 512],
                            start=(cc == 0), stop=(cc == 1))
                    sn = rp.tile([1, 512], F32, tag="sn")
                    nc.scalar.activation(sn[:], nsq[:], AF.Sqrt)
                    rn = rp.tile([1, 512], F32, tag="rn")
                    nc.vector.reciprocal(rn[:], sn[:])
                    bcp = ppB.tile([16, 512], F32, tag="bcp")
                    nc.tensor.matmul(bcp[:], lhsT=ones_r[:1, :16], rhs=rn[:],
                                     start=True, stop=True)
                    dsb = rp.tile([16, 512], F32, tag="dsb")
                    nc.vector.tensor_copy(dsb[:], dots[:])
                    q = rp.tile([128, 512], F32, tag="q")
                    nc.vector.memset(q[:], 4.0)
                    nc.vector.tensor_tensor(q[:16, :], dsb[:], bcp[:], OP.mult)
                    # dist = sqrt(max(2 - 2q, 0)); rows 16.. stay 4.0 (unused)
                    nc.vector.tensor_scalar(q[:16, :], q[:16, :], -2.0, 2.0,
                                            op0=OP.mult, op1=OP.add)
                    nc.vector.tensor_scalar_max(q[:16, :], q[:16, :], 0.0)
                    nc.scalar.activation(q[:], q[:], AF.Sqrt)
                    for j in range(4):
                        i = tt * 4 + j
                        tp = ppB2.tile([128, 128], F32, tag="ttp")
                        nc.tensor.transpose(tp[:], q[:, j * 128:(j + 1) * 128],
                                            ident[:])
                        nc.vector.tensor_copy(distTM[:, i, :], tp[:, :16])

                # softmax + top2 + slots, bulk over all 16 tiles
                ppB2_cm.__exit__(None, None, None)
                ppB_cm.__exit__(None, None, None)
                ppC_cm = tc.tile_pool(name="ppC", bufs=2, space="PSUM")
                ppC = ppC_cm.__enter__()
                cum_sb = spool.tile([128, TILES, E], F32)
                mask_sb = spool.tile([128, TILES, E], F32)

                exa = rp.tile([128, TILES, E], F32, tag="exa")
                nc.scalar.activation(exa[:], distTM[:], AF.Exp, scale=-1.0)
                ssum = rp.tile([128, TILES], F32, tag="ssum")
                nc.vector.tensor_reduce(ssum[:], exa[:],
                                        mybir.AxisListType.X, OP.add)
                rsum = rp.tile([128, TILES], F32, tag="rsum")
                nc.vector.reciprocal(rsum[:], ssum[:])
                nc.vector.tensor_tensor(
                    probs_all[:], exa[:],
                    rsum[:, :, None].to_broadcast([128, TILES, E]), OP.mult)
                mx8a = rp.tile([128, TILES, 8], F32, tag="mx8a")
                for i in range(TILES):
                    nc.vector.max(mx8a[:, i, :], probs_all[:, i, :])
                nc.vector.tensor_copy(w01[:], mx8a[:, :, 0:2])
                nc.vector.tensor_tensor(
                    mask_sb[:], probs_all[:],
                    mx8a[:, :, 1:2].to_broadcast([128, TILES, E]), OP.is_ge)
                comb = rp.tile([128, TILES, E], F32, tag="comb")
                nc.vector.tensor_tensor(comb[:], probs_all[:], mask_sb[:],
                                        OP.mult)
                s1 = rp.tile([128, TILES], F32, tag="s1")
                nc.vector.tensor_reduce(s1[:], comb[:],
                                        mybir.AxisListType.X, OP.add)
                nc.sync.dma_start(
                    sx_sb[:], xs_d[:].rearrange("(i p) h -> p i h", p=128))
                nc.vector.tensor_tensor(
                    sx_sb[:], sx_sb[:],
                    s1[:, :, None].to_broadcast([128, TILES, H]), OP.mult)

                # inclusive cumsum over tokens within each tile (bulk)
                cump = ppC.tile([128, TILES * E], F32, tag="cump")
                nc.tensor.matmul(
                    cump[:], lhsT=ut128[:],
                    rhs=mask_sb[:].rearrange("p i e -> p (i e)"),
                    start=True, stop=True)
                nc.vector.tensor_copy(
                    cum_sb[:].rearrange("p i e -> p (i e)"), cump[:])

                nc.sync.dma_start(
                    dprob_d[:].rearrange("(i p) e -> p i e", p=128),
                    probs_all[:])

                # inter-tile exclusive offsets
                totd = dram.tile([TILES * E], F32)
                nc.sync.dma_start(
                    totd[:].rearrange("(o f) -> o f", o=1),
                    cum_sb[127:128].rearrange("o i e -> o (i e)"))
                tot = rp.tile([16, E], F32, tag="tot")
                nc.sync.dma_start(tot[:],
                                  totd[:].rearrange("(i e) -> i e", i=TILES))
                texp = ppC.tile([16, E], F32, tag="texp")
                nc.tensor.matmul(texp[:], lhsT=sut128[:16, :16], rhs=tot[:],
                                 start=True, stop=True)
                texc = rp.tile([16, E], F32, tag="texc")
                nc.vector.tensor_copy(texc[:], texp[:])
                texd = dram.tile([TILES * E], F32)
                nc.sync.dma_start(
                    texd[:].rearrange("(i e) -> i e", i=TILES), texc[:])
                texr = rp.tile([1, TILES * E], F32, tag="texr")
                nc.sync.dma_start(texr[:], texd[:][None, :])
                bcp2 = ppC.tile([128, TILES * E], F32, tag="bcp2")
                nc.tensor.matmul(bcp2[:], lhsT=ones_r[:1, :], rhs=texr[:1, :],
                                 start=True, stop=True)

                gl = rp.tile([128, TILES, E], F32, tag="gl")   # local slot
                nc.vector.tensor_tensor(
                    gl[:].rearrange("p i e -> p (i e)"),
                    cum_sb[:].rearrange("p i e -> p (i e)"), bcp2[:], OP.add)
                nc.vector.tensor_scalar_add(gl[:], gl[:], -1.0)
                gi = rp.tile([128, TILES, E], I32, tag="gi")
                nc.vector.tensor_copy(gi[:], gl[:])
                g = rp.tile([128, TILES, E], F32, tag="g")     # global slot
                gint = rp.tile([128, TILES, E], I32, tag="gint")
                nc.vector.tensor_tensor(gint[:], gi[:], ebase_i[:], OP.add)
                nc.vector.tensor_copy(g[:], gint[:])
                # wrapped scatter offset f = e*CAP + (s%16)*24 + s//16
                sri = rp.tile([128, TILES, E], I32, tag="sri")
                nc.vector.tensor_scalar(sri[:], gi[:], 15, None,
                                        op0=OP.bitwise_and)
                sci = rp.tile([128, TILES, E], I32, tag="sci")
                nc.vector.tensor_scalar(sci[:], gi[:], 4, None,
                                        op0=OP.logical_shift_right)
                f1 = rp.tile([128, TILES, E], I32, tag="f1")
                nc.vector.tensor_scalar(f1[:], sri[:], CAP // 16, None,
                                        op0=OP.mult)
                nc.vector.tensor_tensor(f1[:], f1[:], sci[:], OP.add)
                nc.vector.tensor_tensor(f1[:], f1[:], ebase_i[:], OP.add)
                # rank masks
                m0 = rp.tile([128, TILES, E], F32, tag="m0")
                nc.vector.tensor_tensor(
                    m0[:], probs_all[:],
                    mx8a[:, :, 0:1].to_broadcast([128, TILES, E]), OP.is_ge)
                m1 = rp.tile([128, TILES, E], F32, tag="m1")
                nc.vector.tensor_tensor(m1[:], mask_sb[:], m0[:], OP.subtract)
                m0i = rp.tile([128, TILES, E], I32, tag="m0i")
                nc.vector.tensor_copy(m0i[:], m0[:])
                m1i = rp.tile([128, TILES, E], I32, tag="m1i")
                nc.vector.tensor_copy(m1i[:], m1[:])
                fra = {}
                for r_, mk in ((0, m0i), (1, m1i)):
                    sel = rp.tile([128, TILES, E], F32, tag=f"sel{r_}")
                    nc.vector.select(sel[:], mk[:], g[:], big[:])
                    sr = rp.tile([128, TILES], F32, tag=f"sr{r_}")
                    nc.vector.tensor_reduce(sr[:], sel[:],
                                            mybir.AxisListType.X, OP.min)
                    nc.vector.tensor_copy(slots_i[:, :, r_], sr[:])
                    self_f = rp.tile([128, TILES, E], I32, tag=f"self{r_}")
                    nc.vector.select(self_f[:], mk[:], f1[:], bigi[:])
                    frt = rp.tile([128, TILES], I32, tag=f"frt{r_}")
                    nc.vector.tensor_reduce(frt[:], self_f[:],
                                            mybir.AxisListType.X, OP.min)
                    fra[r_] = frt
                for i in range(TILES):
                    for r_ in (0, 1):
                        nc.gpsimd.indirect_dma_start(
                            out=idxp[:],
                            out_offset=bass.IndirectOffsetOnAxis(
                                ap=fra[r_][:, i:i + 1], axis=0),
                            in_=ids16[:, i:i + 1],
                            in_offset=None)
                ppC_cm.__exit__(None, None, None)

            nc.sync.dma_start(
                dslot_d[:].rearrange("(i p) r -> p i r", p=128), slots_i[:])

            # =============== EXPERTS (bf16) ===============
            with tc.tile_pool(name="ex", bufs=2) as ep, \
                 tc.tile_pool(name="exz", bufs=2) as ezp, \
                 tc.tile_pool(name="exp", bufs=2, space="PSUM") as epp:
                for e in range(E):
                    idx128 = ep.tile([128, CAP // 16], I16, tag="idx128")
                    for k8 in range(8):
                        nc.sync.dma_start(idx128[k8 * 16:(k8 + 1) * 16, :],
                                          idxp[e * CAP:(e + 1) * CAP,
                                               0].rearrange("(p s) -> p s",
                                                            p=16))
                    xgT = ep.tile([128, HC, CAP], BF16, tag="xgT")
                    nc.gpsimd.dma_gather(out_ap=xgT[:], in_ap=xb_d[:],
                                         idxs_ap=idx128[:], num_idxs=CAP,
                                         num_idxs_reg=CAP, elem_size=H,
                                         transpose=True)
                    xg = ep.tile([128, CAPT, H], BF16, tag="xg")
                    nc.gpsimd.dma_gather(out_ap=xg[:], in_ap=xb_d[:],
                                         idxs_ap=idx128[:], num_idxs=CAP,
                                         num_idxs_reg=CAP, elem_size=H,
                                         transpose=False)
                    wg_sb = ep.tile([128, HC, H], BF16, tag="wg")
                    nc.sync.dma_start(
                        wg_sb[:],
                        wg_d[e].rearrange("(hc p) j -> p hc j", p=128))
                    u_sb = ep.tile([128, HC, R], BF16, tag="u")
                    nc.sync.dma_start(
                        u_sb[:], u_d[e].rearrange("(hc p) r -> p hc r", p=128))
                    v_sb = ep.tile([128, H], BF16, tag="v")
                    nc.sync.dma_start(v_sb[:], v_d[e])
                    bgf = ep.tile([1, H], F32, tag="bgf")
                    nc.sync.dma_start(bgf[:], bg_d[e][None, :])
                    bgb = ep.tile([1, H], BF16, tag="bgb")
                    nc.vector.tensor_copy(bgb[:], bgf[:])

                    # gate (token-major): sigmoid(xg @ Wg + bg)
                    gate = ep.tile([128, CAPT, H], BF16, tag="gate")
                    for i in range(CAPT):
                        for jh in range(2):
                            gp = epp.tile([128, 512], F32, tag="gp")
                            for hc in range(HC):
                                nc.tensor.matmul(
                                    gp[:],
                                    lhsT=xgT[:, hc, i * 128:(i + 1) * 128],
                                    rhs=wg_sb[:, hc, jh * 512:(jh + 1) * 512],
                                    start=(hc == 0), stop=False)
                            nc.tensor.matmul(
                                gp[:], lhsT=ones_rb[:1, :],
                                rhs=bgb[:1, jh * 512:(jh + 1) * 512],
                                start=False, stop=True)
                            nc.scalar.activation(
                                gate[:, i, jh * 512:(jh + 1) * 512], gp[:],
                                AF.Sigmoid)

                    # trans1^T = U^T xg^T   [128(r), CAP]
                    t1p = epp.tile([128, CAP], F32, tag="t1p")
                    for hc in range(HC):
                        nc.tensor.matmul(t1p[:], lhsT=u_sb[:, hc, :],
                                         rhs=xgT[:, hc, :],
                                         start=(hc == 0), stop=(hc == HC - 1))
                    t1T = ep.tile([128, CAP], BF16, tag="t1T")
                    nc.vector.tensor_copy(t1T[:], t1p[:])

                    # trans (token-major) + z = gate*(trans - xg)
                    z_sb = ezp.tile([128, CAPT, H], BF16, tag="z")
                    for i in range(CAPT):
                        for jh in range(2):
                            vp = epp.tile([128, 512], F32, tag="vp")
                            nc.tensor.matmul(
                                vp[:], lhsT=t1T[:, i * 128:(i + 1) * 128],
                                rhs=v_sb[:, jh * 512:(jh + 1) * 512],
                                start=True, stop=True)
                            sl = slice(jh * 512, (jh + 1) * 512)
                            zt = ep.tile([128, 512], F32, tag="zt")
                            nc.vector.tensor_tensor(zt[:], vp[:], xg[:, i, sl],
                                                    OP.subtract)
                            nc.vector.tensor_tensor(z_sb[:, i, sl], zt[:],
                                                    gate[:, i, sl], OP.mult)
                    nc.sync.dma_start(
                        zp[e * CAP:(e + 1) * CAP, :].rearrange(
                            "(c p) j -> p c j", p=128), z_sb[:])

            # =============== COMBINE ===============
            with tc.tile_pool(name="cb", bufs=3) as cb:
                for i in range(TILES):
                    z0 = cb.tile([128, H], BF16, tag="z0")
                    nc.gpsimd.indirect_dma_start(
                        out=z0[:], out_offset=None, in_=zp[:],
                        in_offset=bass.IndirectOffsetOnAxis(
                            ap=slots_i[:, i, 0:1], axis=0))
                    z1 = cb.tile([128, H], BF16, tag="z1")
                    nc.gpsimd.indirect_dma_start(
                        out=z1[:], out_offset=None, in_=zp[:],
                        in_offset=bass.IndirectOffsetOnAxis(
                            ap=slots_i[:, i, 1:2], axis=0))
                    t0 = cb.tile([128, H], F32, tag="t0")
                    nc.vector.tensor_scalar_mul(t0[:], z0[:], w01[:, i, 0:1])
                    acc = cb.tile([128, H], F32, tag="acc")
                    nc.vector.tensor_tensor(acc[:], t0[:], sx_sb[:, i, :],
                                            OP.add)
                    t1 = cb.tile([128, H], F32, tag="t1")
                    nc.vector.tensor_scalar_mul(t1[:], z1[:], w01[:, i, 1:2])
                    nc.vector.tensor_tensor(acc[:], acc[:], t1[:], OP.add)
                    nc.sync.dma_start(out_d[i * 128:(i + 1) * 128, :], acc[:])

    nc.compile()
    return nc


def _get_built():
    global _BUILT
    if _BUILT is None:
        _BUILT = _build()
    return _BUILT


def make_in_maps(inputs):
    x = np.asarray(inputs["x"], np.float32)
    Wd = np.asarray(inputs["Wd"], np.float32)
    bd = np.asarray(inputs["bd"], np.float32)
    cen = np.asarray(inputs["centroids"], np.float32)
    Wg = np.asarray(inputs["Wg"], np.float32)
    bg = np.asarray(inputs["bg"], np.float32)
    U = np.asarray(inputs["U"], np.float32)
    V = np.asarray(inputs["V"], np.float32)

    xT = np.ascontiguousarray(x.T)
    Wgb = Wg.astype(ml_dtypes.bfloat16)
    Ub = U.astype(ml_dtypes.bfloat16)
    Vb = V.astype(ml_dtypes.bfloat16)

    in_maps = []
    for c in range(NCORES):
        sl = slice(c * NT, (c + 1) * NT)
        xb = np.zeros((NT + 128, H), ml_dtypes.bfloat16)
        xb[:NT] = x[sl].astype(ml_dtypes.bfloat16)
        in_maps.append({
            "xs": x[sl],
            "xT": np.ascontiguousarray(xT[:, sl]),
            "xb": xb,
            "Wd": Wd, "bd": bd, "cen": cen,
            "Wgb": Wgb, "bg": bg, "Ub": Ub, "Vb": Vb,
        })
    return in_maps


def kernel(**inputs):
    nc = _get_built()
    in_maps = make_in_maps(inputs)
    import os
    trace = bool(int(os.environ.get("KERNEL_TRACE", "0")))
    res = run_bass_kernel_spmd(nc, in_maps, list(range(NCORES)),
                               trace=trace)
    out = np.concatenate([res.results[c]["out"] for c in range(NCORES)],
                         axis=0)
    kernel.last_results = res
    return out


if __name__ == "__main__":
    import time
    t0 = time.time()
    _get_built()
    print("built in %.1fs" % (time.time() - t0))


# revision 20
# speedup vs baseline: 3.4846x; 3.4846x over previous
"""MoE routing kernel for Trainium2, 8 NeuronCores, SPMD.

Strategy: token-parallel across cores (2048 tokens/core); each core runs the
router in fp32 (selection is numerically chaotic -- probs are near-uniform, so
the router matmul/distances must be fp32; monotone transforms are skipped where
they don't affect ordering), then computes only the routed (token, expert)
pairs: per expert a static-capacity (384) compacted token list is built on
device (matmul cumsum + indirect scatter), tokens are gathered with
dma_gather (transposed + row-major bf16), expert FFN runs in bf16 on the
tensor engine, and per-token contributions are combined token-major.
"""

import sys
import numpy as np

sys.path.insert(0, "/opt/trn_rl_repo")

import ml_dtypes

import concourse.bass as bass
import concourse.mybir as mybir
import concourse.tile as tile
from concourse import bacc
from concourse.bass_utils import run_bass_kernel_spmd
from concourse.masks import make_identity, make_upper_triangular

F32 = mybir.dt.float32
BF16 = mybir.dt.bfloat16
I32 = mybir.dt.int32
I16 = mybir.dt.int16
AF = mybir.ActivationFunctionType
OP = mybir.AluOpType

N, H, C, E, R = 16384, 1024, 256, 16, 128
NCORES = 8
NT = N // NCORES          # tokens per core = 2048
TILES = NT // 128         # 16 token tiles
CAP = 384                 # per-expert capacity (measured max 339)
CAPT = CAP // 128         # 3 slot tiles
HC = H // 128             # 8 hidden chunks
PADROW = NT               # trash row id in padded xb
NPOOL = E * CAP           # 6144

_BUILT = None


def _build():
    nc = bacc.Bacc("TRN2", target_bir_lowering=False, debug=False,
                   num_devices=NCORES)

    xs_d = nc.dram_tensor("xs", [NT, H], F32, kind="ExternalInput")
    xT_d = nc.dram_tensor("xT", [H, NT], F32, kind="ExternalInput")
    xb_d = nc.dram_tensor("xb", [NT + 128, H], BF16, kind="ExternalInput")
    wd_d = nc.dram_tensor("Wd", [H, C], F32, kind="ExternalInput")
    bd_d = nc.dram_tensor("bd", [C], F32, kind="ExternalInput")
    cen_d = nc.dram_tensor("cen", [E, C], F32, kind="ExternalInput")
    wg_d = nc.dram_tensor("Wgb", [E, H, H], BF16, kind="ExternalInput")
    bg_d = nc.dram_tensor("bg", [E, H], F32, kind="ExternalInput")
    u_d = nc.dram_tensor("Ub", [E, H, R], BF16, kind="ExternalInput")
    v_d = nc.dram_tensor("Vb", [E, R, H], BF16, kind="ExternalInput")

    out_d = nc.dram_tensor("out", [NT + 128, H], F32, kind="ExternalOutput")
    dprob_d = nc.dram_tensor("dbg_probs", [NT, E], F32, kind="ExternalOutput")
    dz_d = nc.dram_tensor("dbg_z0", [CAP, H], F32, kind="ExternalOutput")

    with tile.TileContext(nc) as tc:
        import contextlib
        ctx = contextlib.ExitStack()
        with ctx:
            cpool = ctx.enter_context(tc.tile_pool(name="const", bufs=1))
            spool = ctx.enter_context(tc.tile_pool(name="state", bufs=1))
            dram = ctx.enter_context(tc.tile_pool(name="dram", bufs=1,
                                                  space="DRAM"))

            # ---------------- constants ----------------
            ident = cpool.tile([128, 128], F32)
            make_identity(nc, ident[:])
            ut128 = cpool.tile([128, 128], F32)       # 1 where p <= i
            make_upper_triangular(nc, ut128[:], val=1.0, diag=True)
            sut128 = cpool.tile([128, 128], F32)      # 1 where p < i
            make_upper_triangular(nc, sut128[:], val=1.0, diag=False)
            ones_k = cpool.tile([128, 1], F32)
            nc.vector.memset(ones_k[:], 1.0)
            ones_r = cpool.tile([1, 128], F32)
            nc.vector.memset(ones_r[:], 1.0)
            ones_rb = cpool.tile([1, 128], BF16)
            nc.vector.memset(ones_rb[:], 1.0)
            big = cpool.tile([128, TILES, E], F32)
            nc.vector.memset(big[:], 8.0e6)
            bigi = cpool.tile([128, TILES, E], I32)
            nc.vector.memset(bigi[:], 8000000)
            ebase_i = cpool.tile([128, TILES, E], I32)    # e*CAP per (i,e)
            nc.gpsimd.iota(ebase_i[:], pattern=[[0, TILES], [CAP, E]],
                           base=0, channel_multiplier=0)
            ids16 = cpool.tile([128, TILES], I16)     # token id = i*128+p
            nc.gpsimd.iota(ids16[:], pattern=[[128, TILES]], base=0,
                           channel_multiplier=1)
            pre_wg, pre_u, pre_v, pre_bgb = [], [], [], []
            for e in range(2):
                w_ = cpool.tile([128, HC, H], BF16)
                nc.sync.dma_start(
                    w_[:], wg_d[e].rearrange("(hc p) j -> p hc j", p=128))
                u_ = cpool.tile([128, HC, R], BF16)
                nc.sync.dma_start(
                    u_[:], u_d[e].rearrange("(hc p) r -> p hc r", p=128))
                v_ = cpool.tile([128, H], BF16)
                nc.sync.dma_start(v_[:], v_d[e])
                bf_ = cpool.tile([1, H], F32)
                nc.sync.dma_start(bf_[:], bg_d[e][None, :])
                bb_ = cpool.tile([1, H], BF16)
                nc.vector.tensor_copy(bb_[:], bf_[:])
                pre_wg.append(w_); pre_u.append(u_); pre_v.append(v_)
                pre_bgb.append(bb_)

            # persistent state across phases
            probs_all = spool.tile([128, TILES, E], F32)

            idxp = dram.tile([NPOOL, 1], I16)              # wrapped idx pool
            acc = dram.tile([NT + 128, H], F32)            # output accumulator
            cpad = dram.tile([NT + 128, 64], F32)          # combine w, 256B rows

            # zero the idx pool (pad slots must hold a valid index)
            zini = cpool.tile([128, NPOOL // 128], I16)
            nc.vector.memset(zini[:], PADROW)
            nc.sync.dma_start(
                idxp[:].rearrange("(p s) o -> p (s o)", p=128), zini[:])

            # =============== ROUTER (fp32) ===============
            with tc.tile_pool(name="rt", bufs=1) as rp, \
                 tc.tile_pool(name="rtd", bufs=1) as rtd:
                xtp_cm = tc.tile_pool(name="xtp", bufs=1)
                xtp = xtp_cm.__enter__()
                xT_sb = xtp.tile([128, HC, NT], F32)
                nc.sync.dma_start(
                    xT_sb[:], xT_d[:].rearrange("(hc p) t -> p hc t", p=128))
                wd_sb = rp.tile([128, HC, C], F32, tag="wd")
                nc.sync.dma_start(
                    wd_sb[:], wd_d[:].rearrange("(hc p) c -> p hc c", p=128))
                bdT = rp.tile([128, 2], F32, tag="bd")
                nc.sync.dma_start(bdT[:],
                                  bd_d[:].rearrange("(cc p) -> p cc", p=128))

                # centroid normalization -> cnT [128(c), 2, 16]
                cen_sb = rp.tile([16, C], F32, tag="cen")
                nc.sync.dma_start(cen_sb[:], cen_d[:])
                csq = rp.tile([16, C], F32, tag="csq")
                nc.vector.tensor_tensor(csq[:], cen_sb[:], cen_sb[:], OP.mult)
                cns = rp.tile([16, 1], F32, tag="cns")
                nc.vector.tensor_reduce(cns[:], csq[:], mybir.AxisListType.X,
                                        OP.add)
                cnrt = rp.tile([16, 1], F32, tag="cnrt")
                nc.scalar.activation(cnrt[:], cns[:], AF.Sqrt)
                cnr = rp.tile([16, 1], F32, tag="cnr")
                nc.vector.reciprocal(cnr[:], cnrt[:])
                cnpad = rp.tile([128, C], F32, tag="cnp")
                nc.vector.memset(cnpad[:], 0.0)
                nc.vector.tensor_scalar_mul(cnpad[:16, :], cen_sb[:], cnr[:])
                ppA_cm = tc.tile_pool(name="ppA", bufs=2, space="PSUM")
                ppA = ppA_cm.__enter__()
                cnT = rp.tile([128, 2, 16], F32, tag="cnT")
                for cc in range(2):
                    tp = ppA.tile([128, 128], F32, tag="ctp")
                    nc.tensor.transpose(tp[:], cnpad[:, cc * 128:(cc + 1) * 128],
                                        ident[:])
                    nc.vector.tensor_copy(cnT[:, cc, :], tp[:, :16])

                # distilled^T = gelu(Wd^T x^T + bd)  [128(c), 2, NT]
                distT = rtd.tile([128, 2, NT], F32)
                for cc in range(2):
                    for tt in range(NT // 512):
                        dp = ppA.tile([128, 512], F32, tag="dp")
                        for hc in range(HC):
                            nc.tensor.matmul(
                                dp[:],
                                lhsT=wd_sb[:, hc, cc * 128:(cc + 1) * 128],
                                rhs=xT_sb[:, hc, tt * 512:(tt + 1) * 512],
                                start=(hc == 0), stop=(hc == HC - 1))
                        nc.scalar.activation(
                            distT[:, cc, tt * 512:(tt + 1) * 512], dp[:],
                            AF.Gelu, bias=bdT[:, cc:cc + 1])

                ppA_cm.__exit__(None, None, None)
                xtp_cm.__exit__(None, None, None)

                # dists token-major [128(t), TILES, 16]
                distTM = spool.tile([128, TILES, E], F32)
                ppB_cm = tc.tile_pool(name="ppB", bufs=1, space="PSUM")
                ppB = ppB_cm.__enter__()
                ppB2_cm = tc.tile_pool(name="ppB2", bufs=2, space="PSUM")
                ppB2 = ppB2_cm.__enter__()
                for tt in range(NT // 512):
                    nsq = ppB.tile([1, 512], F32, tag="nsq")
                    dots = ppB.tile([16, 512], F32, tag="dots")
                    for cc in range(2):
                        sqc = rp.tile([128, 512], F32, tag="sqc")
                        nc.vector.tensor_tensor(
                            sqc[:], distT[:, cc, tt * 512:(tt + 1) * 512],
                            distT[:, cc, tt * 512:(tt + 1) * 512], OP.mult)
                        nc.tensor.matmul(
                            nsq[:], lhsT=ones_k[:],
                            rhs=sqc[:],
                            start=(cc == 0), stop=(cc == 1))
                        nc.tensor.matmul(
                            dots[:], lhsT=cnT[:, cc, :],
                            rhs=distT[:, cc, tt * 512:(tt + 1) * 512],
                            start=(cc == 0), stop=(cc == 1))
                    sn = rp.tile([1, 512], F32, tag="sn")
                    nc.scalar.activation(sn[:], nsq[:], AF.Sqrt)
                    rn = rp.tile([1, 512], F32, tag="rn")
                    nc.vector.reciprocal(rn[:], sn[:])
                    bcp = ppB.tile([16, 512], F32, tag="bcp")
                    nc.tensor.matmul(bcp[:], lhsT=ones_r[:1, :16], rhs=rn[:],
                                     start=True, stop=True)
                    dsb = rp.tile([16, 512], F32, tag="dsb")
                    nc.vector.tensor_copy(dsb[:], dots[:])
                    q = rp.tile([128, 512], F32, tag="q")
                    nc.vector.memset(q[:], 4.0)
                    nc.vector.tensor_tensor(q[:16, :], dsb[:], bcp[:], OP.mult)
                    # dist = sqrt(max(2 - 2q, 0)); rows 16.. stay 4.0 (unused)
                    nc.vector.tensor_scalar(q[:16, :], q[:16, :], -2.0, 2.0,
                                            op0=OP.mult, op1=OP.add)
                    nc.vector.tensor_scalar_max(q[:16, :], q[:16, :], 0.0)
                    nc.scalar.activation(q[:], q[:], AF.Sqrt)
                    for j in range(4):
                        i = tt * 4 + j
                        tp = ppB2.tile([128, 128], F32, tag="ttp")
                        nc.tensor.transpose(tp[:], q[:, j * 128:(j + 1) * 128],
                                            ident[:])
                        nc.vector.tensor_copy(distTM[:, i, :], tp[:, :16])

                # softmax + top2 + slots, bulk over all 16 tiles
                ppB2_cm.__exit__(None, None, None)
                ppB_cm.__exit__(None, None, None)
                ppC_cm = tc.tile_pool(name="ppC", bufs=2, space="PSUM")
                ppC = ppC_cm.__enter__()
                cum_sb = spool.tile([128, TILES, E], F32)
                mask_sb = spool.tile([128, TILES, E], F32)

                sxp_cm = tc.tile_pool(name="sxp", bufs=1)
                sxp = sxp_cm.__enter__()
                sx_sb = sxp.tile([128, TILES, H], F32)
                exa = rp.tile([128, TILES, E], F32, tag="exa")
                nc.scalar.activation(exa[:], distTM[:], AF.Exp, scale=-1.0)
                ssum = rp.tile([128, TILES], F32, tag="ssum")
                nc.vector.tensor_reduce(ssum[:], exa[:],
                                        mybir.AxisListType.X, OP.add)
                rsum = rp.tile([128, TILES], F32, tag="rsum")
                nc.vector.reciprocal(rsum[:], ssum[:])
                nc.vector.tensor_tensor(
                    probs_all[:], exa[:],
                    rsum[:, :, None].to_broadcast([128, TILES, E]), OP.mult)
                mx8a = rp.tile([128, TILES, 8], F32, tag="mx8a")
                for i in range(TILES):
                    nc.vector.max(mx8a[:, i, :], probs_all[:, i, :])

                nc.vector.tensor_tensor(
                    mask_sb[:], probs_all[:],
                    mx8a[:, :, 1:2].to_broadcast([128, TILES, E]), OP.is_ge)
                comb = rp.tile([128, TILES, E], F32, tag="comb")
                nc.vector.tensor_tensor(comb[:], probs_all[:], mask_sb[:],
                                        OP.mult)
                s1 = rp.tile([128, TILES], F32, tag="s1")
                nc.vector.tensor_reduce(s1[:], comb[:],
                                        mybir.AxisListType.X, OP.add)
                nc.sync.dma_start(
                    cpad[:NT, :E].rearrange("(i p) e -> p i e", p=128),
                    comb[:])
                nc.sync.dma_start(
                    sx_sb[:], xs_d[:].rearrange("(i p) h -> p i h", p=128))
                nc.vector.tensor_tensor(
                    sx_sb[:], sx_sb[:],
                    s1[:, :, None].to_broadcast([128, TILES, H]), OP.mult)
                nc.sync.dma_start(
                    acc[:NT].rearrange("(i p) h -> p i h", p=128), sx_sb[:])

                # inclusive cumsum over tokens within each tile (bulk)
                cump = ppC.tile([128, TILES * E], F32, tag="cump")
                nc.tensor.matmul(
                    cump[:], lhsT=ut128[:],
                    rhs=mask_sb[:].rearrange("p i e -> p (i e)"),
                    start=True, stop=True)
                nc.vector.tensor_copy(
                    cum_sb[:].rearrange("p i e -> p (i e)"), cump[:])

                nc.sync.dma_start(
                    dprob_d[:].rearrange("(i p) e -> p i e", p=128),
                    probs_all[:])

                # inter-tile exclusive offsets
                totd = dram.tile([TILES * E], F32)
                nc.sync.dma_start(
                    totd[:].rearrange("(o f) -> o f", o=1),
                    cum_sb[127:128].rearrange("o i e -> o (i e)"))
                tot = rp.tile([16, E], F32, tag="tot")
                nc.sync.dma_start(tot[:],
                                  totd[:].rearrange("(i e) -> i e", i=TILES))
                texp = ppC.tile([16, E], F32, tag="texp")
                nc.tensor.matmul(texp[:], lhsT=sut128[:16, :16], rhs=tot[:],
                                 start=True, stop=True)
                texc = rp.tile([16, E], F32, tag="texc")
                nc.vector.tensor_copy(texc[:], texp[:])
                texd = dram.tile([TILES * E], F32)
                nc.sync.dma_start(
                    texd[:].rearrange("(i e) -> i e", i=TILES), texc[:])
                texr = rp.tile([1, TILES * E], F32, tag="texr")
                nc.sync.dma_start(texr[:], texd[:][None, :])
                bcp2 = ppC.tile([128, TILES * E], F32, tag="bcp2")
                nc.tensor.matmul(bcp2[:], lhsT=ones_r[:1, :], rhs=texr[:1, :],
                                 start=True, stop=True)

                gl = rp.tile([128, TILES, E], F32, tag="gl")   # local slot
                nc.vector.tensor_tensor(
                    gl[:].rearrange("p i e -> p (i e)"),
                    cum_sb[:].rearrange("p i e -> p (i e)"), bcp2[:], OP.add)
                nc.vector.tensor_scalar_add(gl[:], gl[:], -1.0)
                gi = rp.tile([128, TILES, E], I32, tag="gi")
                nc.vector.tensor_copy(gi[:], gl[:])
                # wrapped scatter offset f = e*CAP + (s%16)*24 + s//16
                sri = rp.tile([128, TILES, E], I32, tag="sri")
                nc.vector.tensor_scalar(sri[:], gi[:], 15, None,
                                        op0=OP.bitwise_and)
                sci = rp.tile([128, TILES, E], I32, tag="sci")
                nc.vector.tensor_scalar(sci[:], gi[:], 4, None,
                                        op0=OP.logical_shift_right)
                f1 = rp.tile([128, TILES, E], I32, tag="f1")
                nc.vector.tensor_scalar(f1[:], sri[:], CAP // 16, None,
                                        op0=OP.mult)
                nc.vector.tensor_tensor(f1[:], f1[:], sci[:], OP.add)
                nc.vector.tensor_tensor(f1[:], f1[:], ebase_i[:], OP.add)
                # rank masks
                m0 = rp.tile([128, TILES, E], F32, tag="m0")
                nc.vector.tensor_tensor(
                    m0[:], probs_all[:],
                    mx8a[:, :, 0:1].to_broadcast([128, TILES, E]), OP.is_ge)
                m1 = rp.tile([128, TILES, E], F32, tag="m1")
                nc.vector.tensor_tensor(m1[:], mask_sb[:], m0[:], OP.subtract)
                m0i = rp.tile([128, TILES, E], I32, tag="m0i")
                nc.vector.tensor_copy(m0i[:], m0[:])
                m1i = rp.tile([128, TILES, E], I32, tag="m1i")
                nc.vector.tensor_copy(m1i[:], m1[:])
                fra = {}
                for r_, mk in ((0, m0i), (1, m1i)):
                    self_f = rp.tile([128, TILES, E], I32, tag=f"self{r_}")
                    nc.vector.select(self_f[:], mk[:], f1[:], bigi[:])
                    frt = rp.tile([128, TILES], I32, tag=f"frt{r_}")
                    nc.vector.tensor_reduce(frt[:], self_f[:],
                                            mybir.AxisListType.X, OP.min)
                    fra[r_] = frt
                for i in range(TILES):
                    for r_ in (0, 1):
                        nc.gpsimd.indirect_dma_start(
                            out=idxp[:],
                            out_offset=bass.IndirectOffsetOnAxis(
                                ap=fra[r_][:, i:i + 1], axis=0),
                            in_=ids16[:, i:i + 1],
                            in_offset=None)
                ppC_cm.__exit__(None, None, None)
                sxp_cm.__exit__(None, None, None)

            # =============== EXPERTS (bf16) ===============
            with tc.tile_pool(name="ex", bufs=2) as ep, \
                 tc.tile_pool(name="exz", bufs=2) as ezp, \
                 tc.tile_pool(name="exp", bufs=2, space="PSUM") as epp:
                for e in range(E):
                    idx128 = ep.tile([128, CAP // 16], I16, tag="idx128")
                    for k8 in range(8):
                        nc.sync.dma_start(idx128[k8 * 16:(k8 + 1) * 16, :],
                                          idxp[e * CAP:(e + 1) * CAP,
                                               0].rearrange("(p s) -> p s",
                                                            p=16))
                    xgT = ep.tile([128, HC, CAP], BF16, tag="xgT")
                    nc.gpsimd.dma_gather(out_ap=xgT[:], in_ap=xb_d[:],
                                         idxs_ap=idx128[:], num_idxs=CAP,
                                         num_idxs_reg=CAP, elem_size=H,
                                         transpose=True)
                    xg = ep.tile([128, CAPT, H], BF16, tag="xg")
                    nc.gpsimd.dma_gather(out_ap=xg[:], in_ap=xb_d[:],
                                         idxs_ap=idx128[:], num_idxs=CAP,
                                         num_idxs_reg=CAP, elem_size=H,
                                         transpose=False)
                    if e < 2:
                        wg_sb, u_sb, v_sb, bgb = (pre_wg[e], pre_u[e],
                                                  pre_v[e], pre_bgb[e])
                    else:
                        wg_sb = ep.tile([128, HC, H], BF16, tag="wg")
                        nc.sync.dma_start(
                            wg_sb[:],
                            wg_d[e].rearrange("(hc p) j -> p hc j", p=128))
                        u_sb = ep.tile([128, HC, R], BF16, tag="u")
                        nc.sync.dma_start(
                            u_sb[:],
                            u_d[e].rearrange("(hc p) r -> p hc r", p=128))
                        v_sb = ep.tile([128, H], BF16, tag="v")
                        nc.sync.dma_start(v_sb[:], v_d[e])
                        bgf = ep.tile([1, H], F32, tag="bgf")
                        nc.sync.dma_start(bgf[:], bg_d[e][None, :])
                        bgb = ep.tile([1, H], BF16, tag="bgb")
                        nc.vector.tensor_copy(bgb[:], bgf[:])

                    # gate (token-major): sigmoid(xg @ Wg + bg)
                    gate = ep.tile([128, CAPT, H], BF16, tag="gate")
                    for i in range(CAPT):
                        for jh in range(2):
                            gp = epp.tile([128, 512], F32, tag="gp")
                            for hc in range(HC):
                                nc.tensor.matmul(
                                    gp[:],
                                    lhsT=xgT[:, hc, i * 128:(i + 1) * 128],
                                    rhs=wg_sb[:, hc, jh * 512:(jh + 1) * 512],
                                    start=(hc == 0), stop=False)
                            nc.tensor.matmul(
                                gp[:], lhsT=ones_rb[:1, :],
                                rhs=bgb[:1, jh * 512:(jh + 1) * 512],
                                start=False, stop=True)
                            nc.scalar.activation(
                                gate[:, i, jh * 512:(jh + 1) * 512], gp[:],
                                AF.Sigmoid)

                    # trans1^T = U^T xg^T   [128(r), CAP]
                    t1p = epp.tile([128, CAP], F32, tag="t1p")
                    for hc in range(HC):
                        nc.tensor.matmul(t1p[:], lhsT=u_sb[:, hc, :],
                                         rhs=xgT[:, hc, :],
                                         start=(hc == 0), stop=(hc == HC - 1))
                    t1T = ep.tile([128, CAP], BF16, tag="t1T")
                    nc.vector.tensor_copy(t1T[:], t1p[:])

                    wsl = ep.tile([128, CAPT, 64], F32, tag="wsl")
                    nc.gpsimd.dma_gather(out_ap=wsl[:], in_ap=cpad[:],
                                         idxs_ap=idx128[:], num_idxs=CAP,
                                         num_idxs_reg=CAP, elem_size=64,
                                         transpose=False)
                    # trans (token-major) + z = w * gate*(trans - xg)
                    z_sb = ezp.tile([128, CAPT, H], F32, tag="z")
                    for i in range(CAPT):
                        for jh in range(2):
                            vp = epp.tile([128, 512], F32, tag="vp")
                            nc.tensor.matmul(
                                vp[:], lhsT=t1T[:, i * 128:(i + 1) * 128],
                                rhs=v_sb[:, jh * 512:(jh + 1) * 512],
                                start=True, stop=True)
                            sl = slice(jh * 512, (jh + 1) * 512)
                            zt = ep.tile([128, 512], F32, tag="zt")
                            nc.vector.tensor_tensor(zt[:], vp[:], xg[:, i, sl],
                                                    OP.subtract)
                            nc.vector.tensor_tensor(z_sb[:, i, sl], zt[:],
                                                    gate[:, i, sl], OP.mult)
                    nc.vector.tensor_tensor(
                        z_sb[:], z_sb[:],
                        wsl[:, :, e:e + 1].to_broadcast([128, CAPT, H]),
                        OP.mult)
                    if e == 0:
                        nc.sync.dma_start(
                            dz_d[:].rearrange("(c p) j -> p c j", p=128),
                            z_sb[:])
                    nc.gpsimd.dma_scatter_add(
                        out_ap=acc[:], in_ap=z_sb[:], idxs_ap=idx128[:],
                        num_idxs=CAP, num_idxs_reg=CAP, elem_size=H)

            nc.sync.dma_start(out_d[:NT], acc[:NT])

    nc.compile()
    return nc


def _get_built():
    global _BUILT
    if _BUILT is None:
        _BUILT = _build()
    return _BUILT


def make_in_maps(inputs):
    x = np.asarray(inputs["x"], np.float32)
    Wd = np.asarray(inputs["Wd"], np.float32)
    bd = np.asarray(inputs["bd"], np.float32)
    cen = np.asarray(inputs["centroids"], np.float32)
    Wg = np.asarray(inputs["Wg"], np.float32)
    bg = np.asarray(inputs["bg"], np.float32)
    U = np.asarray(inputs["U"], np.float32)
    V = np.asarray(inputs["V"], np.float32)

    xT = np.ascontiguousarray(x.T)
    Wgb = Wg.astype(ml_dtypes.bfloat16)
    Ub = U.astype(ml_dtypes.bfloat16)
    Vb = V.astype(ml_dtypes.bfloat16)

    in_maps = []
    for c in range(NCORES):
        sl = slice(c * NT, (c + 1) * NT)
        xb = np.zeros((NT + 128, H), ml_dtypes.bfloat16)
        xb[:NT] = x[sl].astype(ml_dtypes.bfloat16)
        in_maps.append({
            "xs": x[sl],
            "xT": np.ascontiguousarray(xT[:, sl]),
            "xb": xb,
            "Wd": Wd, "bd": bd, "cen": cen,
            "Wgb": Wgb, "bg": bg, "Ub": Ub, "Vb": Vb,
        })
    return in_maps


def kernel(**inputs):
    nc = _get_built()
    in_maps = make_in_maps(inputs)
    import os
    trace = bool(int(os.environ.get("KERNEL_TRACE", "0")))
    res = run_bass_kernel_spmd(nc, in_maps, list(range(NCORES)),
                               trace=trace)
    out = np.concatenate([res.results[c]["out"][:NT] for c in range(NCORES)],
                         axis=0)
    kernel.last_results = res
    return out


if __name__ == "__main__":
    import time
    t0 = time.time()
    _get_built()
    print("built in %.1fs" % (time.time() - t0))
